# revision 6
# baseline (speedup 1.0000x reference)
"""Trainium2 kernel for nn_CascadedABCDCircuit: cascaded 2-port ABCD ladder.

Math: each stage multiplies the ABCD state by (I + s_i*G_i) where G_i is a
constant nilpotent 2x2 complex matrix and s_i = (omega*v_i)^{+-1} (the
complex reciprocals 1/(w*(1/Q + j)) are just const/w). So every output
component is a Laurent polynomial in omega, degree -6..+6, with
batch-dependent coefficients. Host computes the 13 coefficients per
(component, batch) exactly in fp64 via the recurrence applied to polynomial
coefficient vectors (tiny (1024,13) complex ops). The device evaluates
out[c,b,f] = sum_m C[c,b,m] * W[m,f] as K=13 matmuls and streams the 256MB
result to HBM — memory-bound, as this problem's regime demands.

Precision/speed: PE fp32 matmul = 4 cyc/col; fp32r = 1 cyc/col but
truncates operands to ~12 mantissa bits. We split both operands into
hi (11-bit) + lo parts and evaluate all four cross terms in a SINGLE
K=52 fp32r matmul by stacking the splits along the contraction dim:
lhsT = [C1;C1;C2;C2], rhs = [W1;W2;W1;W2]. Matmul throughput is per
moving column (K only fills the array), so the corrections are free:
full fp32-grade precision at 1 cyc/col and 1 LDWEIGHTS per tile.

Sharding: pure data-parallel over batch: 8 cores x 128 batches, every core
sees all 8192 freqs. Per-core input is a single [52, 1024 + 8192] tensor
(stacked coefficient blocks in lhsT layout, then stacked W rows), loaded
in pieces so the first matmul only waits on its own slices. Output DMAs
are spread round-robin over several hardware DGE queues.
"""
import numpy as np
import sys

for _p in ("/opt/trn_rl_repo", "/root/.axon_site/_ro/trn_rl_repo"):
    if _p not in sys.path:
        sys.path.append(_p)

import concourse.bacc as bacc
import concourse.mybir as mybir
from concourse import tile
from concourse.bass_utils import run_bass_kernel_spmd

# Problem constants (hardcoded per contract)
B, F = 1024, 8192
OP_CODES = [3, 0, 1, 2, 3, 0, 1, 2, 3, 0, 1, 2]
Q_L, Q_C = 50.0, 100.0
NK, K0 = 13, 6               # omega powers -6..+6
NCORES = 8
BPC = B // NCORES            # 128 batches per core
NCOMP = 8                    # Ar, Ai, Br, Bi, Cr, Ci, Dr, Di
OM0 = 2.0 * np.pi * np.sqrt(1e9 * 10e9)   # omega normalizer (geometric mid)

PS_CHUNK = 2048              # psum tile free dim (4 banks)
MM_N = 512                   # moving free dim per matmul (1 PSUM bank, fp32)
HI_BITS = 11                 # mantissa bits kept in the hi split (fp32r-safe)
KS = 4 * NK                  # stacked contraction dim (52)

CB = NCOMP * BPC             # coefficient columns (1024)

LAST_RESULTS = None          # BassKernelResults of the most recent run
_COMPILED = {}


def _round_keep(x, t):
    """Round fp32 array to t mantissa bits (round-to-nearest on the kept bits)."""
    b = np.ascontiguousarray(x, np.float32).view(np.uint32).copy()
    shift = np.uint32(23 - t)
    add = np.uint32(1 << (23 - t - 1))
    b2 = ((b + add) >> shift) << shift
    return b2.view(np.float32)


def _host_coeffs(values):
    """values (B,12) fp32 -> (NCOMP, B, NK) fp64 coeffs in powers of (om/OM0)."""
    v = values.astype(np.float64)
    nb = v.shape[0]
    A = np.zeros((nb, NK), np.complex128); A[:, K0] = 1.0
    Bm = np.zeros((nb, NK), np.complex128)
    Cm = np.zeros((nb, NK), np.complex128)
    Dm = np.zeros((nb, NK), np.complex128); Dm[:, K0] = 1.0

    def shift_mul(P, fac, dk):
        out = np.zeros_like(P)
        if dk == 1:
            out[:, 1:] = P[:, :-1]
        else:
            out[:, :-1] = P[:, 1:]
        return out * fac[:, None]

    for i, code in enumerate(OP_CODES):
        vi = v[:, i]
        if code == 0:      # series L
            fac = vi * OM0 * (1.0 / Q_L + 1j)
            Bm = Bm + shift_mul(A, fac, +1)
            Dm = Dm + shift_mul(Cm, fac, +1)
        elif code == 1:    # series C (reciprocal)
            c = (1.0 / Q_C - 1j) / (1.0 + 1.0 / Q_C**2)
            fac = c / (vi * OM0)
            Bm = Bm + shift_mul(A, fac, -1)
            Dm = Dm + shift_mul(Cm, fac, -1)
        elif code == 2:    # shunt L (reciprocal)
            c = (1.0 / Q_L - 1j) / (1.0 + 1.0 / Q_L**2)
            fac = c / (vi * OM0)
            A = A + shift_mul(Bm, fac, -1)
            Cm = Cm + shift_mul(Dm, fac, -1)
        else:              # shunt C
            fac = vi * OM0 * (1.0 / Q_C + 1j)
            A = A + shift_mul(Bm, fac, +1)
            Cm = Cm + shift_mul(Dm, fac, +1)
    return np.stack([A.real, A.imag, Bm.real, Bm.imag,
                     Cm.real, Cm.imag, Dm.real, Dm.imag])


def _build_module():
    """SPMD module: cw[52, Cstack|Wstack] -> out[NCOMP, BPC, F]."""
    nc = bacc.Bacc("TRN2", target_bir_lowering=False, debug=False,
                   num_devices=NCORES)
    cw_cols = CB + F
    cw_d = nc.dram_tensor("cw", [KS, cw_cols], mybir.dt.float32r,
                          kind="ExternalInput")
    out_d = nc.dram_tensor("out", [NCOMP, BPC, F], mybir.dt.float32,
                           kind="ExternalOutput")

    with tile.TileContext(nc) as tc:
        with (
            tc.tile_pool(name="const", bufs=1) as cpool,
            tc.tile_pool(name="stage", bufs=4) as spool,
            tc.tile_pool(name="ps", bufs=2, space="PSUM") as pspool,
        ):
            cw = cpool.tile([KS, cw_cols], mybir.dt.float32r)
            # load coefficients first, then W in chunks, so the first
            # matmuls only wait on what they read
            nc.sync.dma_start(cw[:, :CB], cw_d[:, :CB])
            for ch in range(F // PS_CHUNK):
                lo = CB + ch * PS_CHUNK
                eng = nc.scalar if ch % 2 else nc.sync
                eng.dma_start(cw[:, lo:lo + PS_CHUNK],
                              cw_d[:, lo:lo + PS_CHUNK])
            ncopy = 0
            # spread output DMAs across distinct HW DGE queues
            dma_engines = [nc.sync, nc.scalar]
            for c in range(NCOMP):
                lhsT = cw[:, c * BPC:(c + 1) * BPC]
                for ch in range(F // PS_CHUNK):
                    acc = pspool.tile([BPC, PS_CHUNK], mybir.dt.float32)
                    ot = spool.tile([BPC, PS_CHUNK], mybir.dt.float32)
                    for j in range(PS_CHUNK // MM_N):
                        col = CB + ch * PS_CHUNK + j * MM_N
                        nc.tensor.matmul(acc[:, j * MM_N:(j + 1) * MM_N],
                                         lhsT, cw[:, col:col + MM_N])
                    # alternate copy engines so neither becomes the bottleneck
                    if ncopy % 2 == 0:
                        nc.vector.tensor_copy(ot[:], acc[:])
                    else:
                        nc.scalar.copy(ot[:], acc[:])
                    dma_engines[ncopy % len(dma_engines)].dma_start(
                        out_d[c, :, ch * PS_CHUNK:(ch + 1) * PS_CHUNK], ot[:])
                    ncopy += 1
    nc.compile()
    return nc


def kernel(values: np.ndarray, freq_hz: np.ndarray) -> np.ndarray:
    global LAST_RESULTS
    assert values.shape == (B, len(OP_CODES)) and freq_hz.shape == (F,)

    # Host precompute (tiny, fp64-exact): Laurent coefficients + omega powers
    coef = _host_coeffs(values)                              # (8, B, 13) f64
    om = 2.0 * np.pi * freq_hz.astype(np.float64)
    wt = om / OM0
    W = np.stack([wt ** (k - K0) for k in range(NK)]).astype(np.float32)
    W1 = _round_keep(W, HI_BITS)
    W2 = (W - W1).astype(np.float32)
    Wstack = np.concatenate([W1, W2, W1, W2])                # (52, F)

    if "nc" not in _COMPILED:
        _COMPILED["nc"] = _build_module()
    nc = _COMPILED["nc"]

    in_maps = []
    for core in range(NCORES):
        sl = slice(core * BPC, (core + 1) * BPC)
        lhs = np.ascontiguousarray(
            np.transpose(coef[:, sl, :], (0, 2, 1))          # (8, 13, BPC)
        ).astype(np.float32)
        cwnp = np.empty((KS, CB + F), np.float32)
        for c in range(NCOMP):
            h = _round_keep(lhs[c], HI_BITS)
            lo = (lhs[c] - h).astype(np.float32)
            blk = cwnp[:, c * BPC:(c + 1) * BPC]
            blk[0 * NK:1 * NK] = h
            blk[1 * NK:2 * NK] = h
            blk[2 * NK:3 * NK] = lo
            blk[3 * NK:4 * NK] = lo
        cwnp[:, CB:] = Wstack
        in_maps.append({"cw": cwnp})

    res = run_bass_kernel_spmd(nc, in_maps, core_ids=list(range(NCORES)))
    LAST_RESULTS = res
    out = np.concatenate([res.results[c]["out"] for c in range(NCORES)], axis=1)
    return out.astype(np.float32, copy=False)


# revision 7
# speedup vs baseline: 1.0159x; 1.0159x over previous
"""Trainium2 kernel for nn_CascadedABCDCircuit: cascaded 2-port ABCD ladder.

Math: each stage multiplies the ABCD state by (I + s_i*G_i) where G_i is a
constant nilpotent 2x2 complex matrix and s_i = (omega*v_i)^{+-1} (the
complex reciprocals 1/(w*(1/Q + j)) are just const/w). So every output
component is a Laurent polynomial in omega, degree -6..+6, with
batch-dependent coefficients. Host computes the 13 coefficients per
(component, batch) exactly in fp64 via the recurrence applied to polynomial
coefficient vectors (tiny (1024,13) complex ops). The device evaluates
out[c,b,f] = sum_m C[c,b,m] * W[m,f] as K=13 matmuls and streams the 256MB
result to HBM — memory-bound, as this problem's regime demands.

Precision/speed: PE fp32 matmul = 4 cyc/col; fp32r = 1 cyc/col but
truncates operands to ~12 mantissa bits. We split both operands into
hi (11-bit) + lo parts and evaluate all four cross terms in a SINGLE
K=52 fp32r matmul by stacking the splits along the contraction dim:
lhsT = [C1;C1;C2], rhs = [W1;W2;W1] (the C2*W2 term is ~2^-24 and
dropped). Matmul throughput is per moving column (K only fills the
array), so the corrections are free: full fp32-grade precision at
1 cyc/col and 1 LDWEIGHTS per tile.

Sharding: pure data-parallel over batch: 8 cores x 128 batches, every core
sees all 8192 freqs. Per-core input is a single [52, 1024 + 8192] tensor
(stacked coefficient blocks in lhsT layout, then stacked W rows), loaded
in pieces so the first matmul only waits on its own slices. Output DMAs
are spread round-robin over several hardware DGE queues.
"""
import numpy as np
import sys

for _p in ("/opt/trn_rl_repo", "/root/.axon_site/_ro/trn_rl_repo"):
    if _p not in sys.path:
        sys.path.append(_p)

import concourse.bacc as bacc
import concourse.mybir as mybir
from concourse import tile
from concourse.bass_utils import run_bass_kernel_spmd

# Problem constants (hardcoded per contract)
B, F = 1024, 8192
OP_CODES = [3, 0, 1, 2, 3, 0, 1, 2, 3, 0, 1, 2]
Q_L, Q_C = 50.0, 100.0
NK, K0 = 13, 6               # omega powers -6..+6
NCORES = 8
BPC = B // NCORES            # 128 batches per core
NCOMP = 8                    # Ar, Ai, Br, Bi, Cr, Ci, Dr, Di
OM0 = 2.0 * np.pi * np.sqrt(1e9 * 10e9)   # omega normalizer (geometric mid)

PS_CHUNK = 2048              # psum tile free dim (4 banks)
MM_N = 512                   # moving free dim per matmul (1 PSUM bank, fp32)
HI_BITS = 11                 # mantissa bits kept in the hi split (fp32r-safe)
KS = 3 * NK                  # stacked contraction dim (39)

CB = NCOMP * BPC             # coefficient columns (1024)

LAST_RESULTS = None          # BassKernelResults of the most recent run
_COMPILED = {}


def _round_keep(x, t):
    """Round fp32 array to t mantissa bits (round-to-nearest on the kept bits)."""
    b = np.ascontiguousarray(x, np.float32).view(np.uint32).copy()
    shift = np.uint32(23 - t)
    add = np.uint32(1 << (23 - t - 1))
    b2 = ((b + add) >> shift) << shift
    return b2.view(np.float32)


def _host_coeffs(values):
    """values (B,12) fp32 -> (NCOMP, B, NK) fp64 coeffs in powers of (om/OM0)."""
    v = values.astype(np.float64)
    nb = v.shape[0]
    A = np.zeros((nb, NK), np.complex128); A[:, K0] = 1.0
    Bm = np.zeros((nb, NK), np.complex128)
    Cm = np.zeros((nb, NK), np.complex128)
    Dm = np.zeros((nb, NK), np.complex128); Dm[:, K0] = 1.0

    def shift_mul(P, fac, dk):
        out = np.zeros_like(P)
        if dk == 1:
            out[:, 1:] = P[:, :-1]
        else:
            out[:, :-1] = P[:, 1:]
        return out * fac[:, None]

    for i, code in enumerate(OP_CODES):
        vi = v[:, i]
        if code == 0:      # series L
            fac = vi * OM0 * (1.0 / Q_L + 1j)
            Bm = Bm + shift_mul(A, fac, +1)
            Dm = Dm + shift_mul(Cm, fac, +1)
        elif code == 1:    # series C (reciprocal)
            c = (1.0 / Q_C - 1j) / (1.0 + 1.0 / Q_C**2)
            fac = c / (vi * OM0)
            Bm = Bm + shift_mul(A, fac, -1)
            Dm = Dm + shift_mul(Cm, fac, -1)
        elif code == 2:    # shunt L (reciprocal)
            c = (1.0 / Q_L - 1j) / (1.0 + 1.0 / Q_L**2)
            fac = c / (vi * OM0)
            A = A + shift_mul(Bm, fac, -1)
            Cm = Cm + shift_mul(Dm, fac, -1)
        else:              # shunt C
            fac = vi * OM0 * (1.0 / Q_C + 1j)
            A = A + shift_mul(Bm, fac, +1)
            Cm = Cm + shift_mul(Dm, fac, +1)
    return np.stack([A.real, A.imag, Bm.real, Bm.imag,
                     Cm.real, Cm.imag, Dm.real, Dm.imag])


def _build_module():
    """SPMD module: cw[52, Cstack|Wstack] -> out[NCOMP, BPC, F]."""
    nc = bacc.Bacc("TRN2", target_bir_lowering=False, debug=False,
                   num_devices=NCORES)
    cw_cols = CB + F
    cw_d = nc.dram_tensor("cw", [KS, cw_cols], mybir.dt.float32r,
                          kind="ExternalInput")
    out_d = nc.dram_tensor("out", [NCOMP, BPC, F], mybir.dt.float32,
                           kind="ExternalOutput")

    with tile.TileContext(nc) as tc:
        with (
            tc.tile_pool(name="const", bufs=1) as cpool,
            tc.tile_pool(name="stage", bufs=4) as spool,
            tc.tile_pool(name="ps", bufs=2, space="PSUM") as pspool,
        ):
            cw = cpool.tile([KS, cw_cols], mybir.dt.float32r)
            # load coefficients first, then W in chunks, so the first
            # matmuls only wait on what they read
            nc.sync.dma_start(cw[:, :CB], cw_d[:, :CB])
            for ch in range(F // PS_CHUNK):
                lo = CB + ch * PS_CHUNK
                eng = nc.scalar if ch % 2 else nc.sync
                eng.dma_start(cw[:, lo:lo + PS_CHUNK],
                              cw_d[:, lo:lo + PS_CHUNK])
            ncopy = 0
            # spread output DMAs across distinct HW DGE queues
            dma_engines = [nc.sync, nc.scalar]
            for c in range(NCOMP):
                lhsT = cw[:, c * BPC:(c + 1) * BPC]
                for ch in range(F // PS_CHUNK):
                    acc = pspool.tile([BPC, PS_CHUNK], mybir.dt.float32)
                    ot = spool.tile([BPC, PS_CHUNK], mybir.dt.float32)
                    for j in range(PS_CHUNK // MM_N):
                        col = CB + ch * PS_CHUNK + j * MM_N
                        nc.tensor.matmul(acc[:, j * MM_N:(j + 1) * MM_N],
                                         lhsT, cw[:, col:col + MM_N])
                    # alternate copy engines so neither becomes the bottleneck
                    if ncopy % 2 == 0:
                        nc.vector.tensor_copy(ot[:], acc[:])
                    else:
                        nc.scalar.copy(ot[:], acc[:])
                    dma_engines[ncopy % len(dma_engines)].dma_start(
                        out_d[c, :, ch * PS_CHUNK:(ch + 1) * PS_CHUNK], ot[:])
                    ncopy += 1
    nc.compile()
    return nc


def kernel(values: np.ndarray, freq_hz: np.ndarray) -> np.ndarray:
    global LAST_RESULTS
    assert values.shape == (B, len(OP_CODES)) and freq_hz.shape == (F,)

    # Host precompute (tiny, fp64-exact): Laurent coefficients + omega powers
    coef = _host_coeffs(values)                              # (8, B, 13) f64
    om = 2.0 * np.pi * freq_hz.astype(np.float64)
    wt = om / OM0
    W = np.stack([wt ** (k - K0) for k in range(NK)]).astype(np.float32)
    W1 = _round_keep(W, HI_BITS)
    W2 = (W - W1).astype(np.float32)
    Wstack = np.concatenate([W1, W2, W1])                    # (39, F)

    if "nc" not in _COMPILED:
        _COMPILED["nc"] = _build_module()
    nc = _COMPILED["nc"]

    in_maps = []
    for core in range(NCORES):
        sl = slice(core * BPC, (core + 1) * BPC)
        lhs = np.ascontiguousarray(
            np.transpose(coef[:, sl, :], (0, 2, 1))          # (8, 13, BPC)
        ).astype(np.float32)
        cwnp = np.empty((KS, CB + F), np.float32)
        for c in range(NCOMP):
            h = _round_keep(lhs[c], HI_BITS)
            lo = (lhs[c] - h).astype(np.float32)
            blk = cwnp[:, c * BPC:(c + 1) * BPC]
            blk[0 * NK:1 * NK] = h
            blk[1 * NK:2 * NK] = h
            blk[2 * NK:3 * NK] = lo
        cwnp[:, CB:] = Wstack
        in_maps.append({"cw": cwnp})

    res = run_bass_kernel_spmd(nc, in_maps, core_ids=list(range(NCORES)))
    LAST_RESULTS = res
    out = np.concatenate([res.results[c]["out"] for c in range(NCORES)], axis=1)
    return out.astype(np.float32, copy=False)
